# revision 25
# baseline (speedup 1.0000x reference)
"""AdaPT quantized linear (int8-exact via fp16 matmul) on 8 TRN2 NeuronCores.

Reference computes:
    qx = clip(round(x * 127/amax_x), -127, 127)        [N, K] int8
    qw = clip(round(w * 127/amax_w), -127, 127)        [M, K] int8
    out = (qx @ qw.T) / ((127/amax_x)*(127/amax_w)) + bias

Strategy: data-parallel over the 8192-token dim (1024 tokens/core), full
weight on every core, no collectives.  All int8 values are exactly
representable in fp16, the PE multiplies fp16 exactly (products < 2^14)
and accumulates in fp32 (partial sums << 2^24), so the fp16 matmul
reproduces the int8 systolic GEMM bit-exactly.

Rounding trick: fp16 has unit spacing on [1024, 2048), so converting
(x*scale + 1536) from fp32 to fp16 rounds the fractional part to the
nearest integer with ties-to-even -- exactly jnp.round.

The PE floor is one fp16 pass = 2048 matmuls x 216ns = 442us (fp8
DoubleRow measured at 2x per k-tile makes any exact 3-product split
1.5x slower -- not viable).  What this version optimizes is everything
around that floor:

  Prologue (chunked partial accumulation): while x streams in (16.8MB,
  ~47us at ~358GB/s), only 4 m-tiles can be PSUM-resident at full token
  width, which caps available PE work below the DMA deficit and used to
  idle the PE ~25us (and drop its pstate clock, doubling matmul time
  for stretches).  Instead the first J=8 m-tiles accumulate k-CHUNKS:
  when x chunk c arrives, each prologue m-tile runs just those k-tiles
  and the PSUM partial (an exact integer in fp32) is spilled/added to
  an SBUF accumulator by the DVE, freeing the bank immediately.  With
  J=8 the per-k-tile balance is PE 3.45us vs DMA 2.93us -- the PE
  stays busy for the whole x stream.  Weight slices for prologue
  m-tiles arrive chunk-by-chunk and die after one wave, so SBUF holds
  only ~1KB/partition of prologue weights at a time.

  Steady state (m-tiles J..31): classic pipeline, weight prep 2 m-tiles
  ahead, unchanged from the previous version.

  Tail: the last m-tile runs its two 512-token banks serially so the
  first bank's dequant+store hides under the second bank's matmuls.

Output is stored as fp16 ([M, tokens/core] transposed per core, so the
bias rides the ScalarE per-partition bias port) and upcast on the host;
the ~3e-4 relative rounding is far inside the 2e-2 gate.
"""

import sys

import numpy as np

sys.path.insert(0, "/opt/trn_rl_repo")

N, K, M = 8192, 4096, 4096
N_CORES = 8
TPC = N // N_CORES  # tokens per core
P = 128
KT = K // P   # 32 k-tiles
MT = M // P   # 32 m-tiles
TF = 512      # matmul moving free dim (one PSUM bank of fp32)
NTF = TPC // TF
MAGIC = 1536.0   # fp16 unit-spacing window base: round via fp32->fp16 convert
MAXV = 127.0
HI = MAGIC + MAXV
LO = MAGIC - MAXV
JUNK_PRE = 32    # PE pstate-ramp warmup matmuls before first real work

J = 8            # prologue m-tiles (chunk-partial accumulation)
# k-tile chunks per prologue wave (pair-aligned).  Chunks must be >=4
# k-tiles: the 16 PSUM->SBUF spills per wave cost ~0.6us each on DVE,
# which only fits under the wave's PE time at >=4 k-tiles per chunk.
CH = [(0, 4), (4, 8), (8, 16), (16, 24), (24, 32)]
NW = len(CH)
# junk plugs after each group in early waves (DMA still catching up)
JW = [1, 1, 0, 0, 0]
# x-chain injection step within a wave, by pending pair count
INJ = {1: (4,), 2: (2, 5), 3: (1, 3, 6), 4: (1, 3, 5, 7)}


def build(s_x: float, s_w: float, inv_s: float):
    import concourse.mybir as mybir
    import concourse.tile as tile
    from concourse import bacc

    dt = mybir.dt
    AF = mybir.ActivationFunctionType
    OP = mybir.AluOpType

    nc = bacc.Bacc("TRN2", target_bir_lowering=False, debug=False,
                   num_devices=N_CORES)

    xt = nc.declare_dram_parameter("xt", [K, TPC], dt.float32, isOutput=False)
    wt = nc.declare_dram_parameter("wt", [MT, K, P], dt.float32, isOutput=False)
    # bias pre-transposed to [P, MT] on the host: the [M]->(p,o) gather
    # was a 4096x4B strided DMA that stalled its issuing queue for 11us
    bias = nc.declare_dram_parameter("bias", [P, MT], dt.float32,
                                     isOutput=False)
    out = nc.declare_dram_parameter("out", [M, TPC], dt.float16, isOutput=True)

    with tile.TileContext(nc) as tc:
        with (
            tc.tile_pool(name="xq", bufs=1) as xq_pool,
            tc.tile_pool(name="xs", bufs=2) as xs_pool,
            tc.tile_pool(name="pwq", bufs=10) as pwq_pool,
            tc.tile_pool(name="pws", bufs=6) as pws_pool,
            tc.tile_pool(name="part", bufs=1) as part_pool,
            tc.tile_pool(name="wsb", bufs=2) as wsb_pool,
            tc.tile_pool(name="wq", bufs=3) as wq_pool,
            tc.tile_pool(name="cst", bufs=1) as cst_pool,
            tc.tile_pool(name="outp", bufs=6) as out_pool,
            tc.tile_pool(name="psA", bufs=4, space="PSUM") as psA_pool,
            tc.tile_pool(name="psB", bufs=3, space="PSUM") as psB_pool,
            tc.tile_pool(name="junk", bufs=1, space="PSUM") as junk_psum_pool,
        ):
            # ---- junk machinery (PE pstate keep-warm) ----------------
            # Junk matmuls write a dedicated PSUM bank that is never
            # read; the input lives outside tile tracking so the first
            # warmup matmuls don't wait on the memset.
            junk_t = nc.alloc_sbuf_tensor("junk_sb", [P, TF], dt.float16)
            junk_sb = junk_t.ap()
            junk_ps = junk_psum_pool.tile([P, TF], dt.float32, name="jps")

            def junk_mms(n):
                for _ in range(n):
                    nc.tensor.matmul(junk_ps[:], junk_sb[:, :P], junk_sb[:],
                                     start=True, stop=False,
                                     skip_group_check=True)

            xq_tiles = [
                xq_pool.tile([P, 2, TPC], dt.float16, name=f"xq{p}",
                             tag=f"xq{p}")
                for p in range(KT // 2)
            ]

            def quant_chain(dst, src, scale, aff_scalar):
                # dst fp16 = clip(round(src*scale), -127, 127) + 1536,
                # recentred; the affine's fp32->fp16 convert is the round.
                if aff_scalar:
                    nc.scalar.activation(dst, src, AF.Copy,
                                         bias=MAGIC, scale=scale)
                else:
                    nc.vector.tensor_scalar(dst, src, scale, MAGIC,
                                            OP.mult, OP.add)
                nc.vector.tensor_scalar(dst, dst, HI, LO, OP.min, OP.max)
                nc.vector.tensor_scalar(dst, dst, MAGIC, None, OP.subtract)

            def issue_x_pair_dma(kp):
                # per-k-tile DMAs (0.5MB arrival quantum)
                kt0 = 2 * kp
                xs = xs_pool.tile([P, 2, TPC], dt.float32, name="xs")
                for h in range(2):
                    nc.sync.dma_start(
                        xs[:, h, :],
                        xt[(kt0 + h) * P:(kt0 + h + 1) * P, :]
                        .rearrange("(o p) t -> p o t", p=P))
                return xs

            def chain_x_pair(kp, xs, aff_scalar=True):
                for h in range(2):
                    quant_chain(xq_tiles[kp][:, h, :], xs[:, h, :], s_x,
                                aff_scalar=aff_scalar)

            # ---- prologue weight slices (one wave's k-chunk, one m-tile)
            def prep_pw(mt, a, b, dma_engine=None):
                ws = pws_pool.tile([P, 8, P], dt.float32, name="pws")
                wqs_ = pwq_pool.tile([P, 8, P], dt.float16, name="pwq")
                (dma_engine or nc.sync).dma_start(
                    ws[:, :b - a, :],
                    wt[mt, a * P:b * P, :]
                    .rearrange("(o p) f -> p o f", p=P))
                # affine on ScalarE (prologue DVE is loaded with spills)
                nc.scalar.activation(wqs_[:, :b - a, :], ws[:, :b - a, :],
                                     AF.Copy, bias=MAGIC, scale=s_w)
                nc.vector.tensor_scalar(wqs_[:, :b - a, :], wqs_[:, :b - a, :],
                                        HI, LO, OP.min, OP.max)
                nc.vector.tensor_scalar(wqs_[:, :b - a, :], wqs_[:, :b - a, :],
                                        MAGIC, None, OP.subtract)
                return wqs_

            def prep_w_big(mt):
                # steady state: 2 half DMAs + 2 half chains on DVE
                wq = wq_pool.tile([P, KT, P], dt.float16, name="wq", tag="wq")
                h = KT // 2
                for q in range(2):
                    ws = wsb_pool.tile([P, KT // 2, P], dt.float32,
                                       name="wsb")
                    nc.sync.dma_start(
                        ws[:],
                        wt[mt, q * h * P:(q + 1) * h * P, :]
                        .rearrange("(o p) f -> p o f", p=P))
                    quant_chain(wq[:, q * h:(q + 1) * h, :], ws[:],
                                s_w, aff_scalar=False)
                return wq

            def prep_w_big_sub(wq, mt, a, b, dma_engine=None):
                # one k-slice of a steady wq tile (prefetch during prologue)
                ws = wsb_pool.tile([P, KT // 2, P], dt.float32, name="wsb")
                (dma_engine or nc.sync).dma_start(
                    ws[:, :b - a, :],
                    wt[mt, a * P:b * P, :]
                    .rearrange("(o p) f -> p o f", p=P))
                quant_chain(wq[:, a:b, :], ws[:, :b - a, :], s_w,
                            aff_scalar=False)

            def store(mt, src_banks, last=False):
                # dequant on ScalarE; out DMA issued from ScalarE too.
                for tf in range(NTF):
                    outt = out_pool.tile([P, TF], dt.float16, name="outt")
                    nc.scalar.activation(
                        outt[:], src_banks[tf][:],
                        AF.Identity, bias=bias_sb[:, mt:mt + 1],
                        scale=inv_s)
                    if last and tf == NTF - 1:
                        h = TF // 2
                        nc.sync.dma_start(
                            out[mt * P:(mt + 1) * P,
                                tf * TF + h:(tf + 1) * TF],
                            outt[:, h:TF])
                        nc.scalar.dma_start(
                            out[mt * P:(mt + 1) * P,
                                tf * TF:tf * TF + h],
                            outt[:, 0:h])
                    else:
                        nc.scalar.dma_start(
                            out[mt * P:(mt + 1) * P,
                                tf * TF:(tf + 1) * TF],
                            outt[:])

            # ---- prologue ---------------------------------------------
            # bias + chunk-0 x pairs + wave-0 weight slices in flight first
            xs0 = xs_pool.tile([P, 2, TPC], dt.float32, name="xs")
            nc.sync.dma_start(
                xs0[:, 0, :], xt[0:P, :].rearrange("(o p) t -> p o t", p=P))
            nc.vector.memset(junk_sb, 0.0)
            nc.sync.dma_start(
                xs0[:, 1, :], xt[P:2 * P, :].rearrange("(o p) t -> p o t", p=P))
            xs1 = issue_x_pair_dma(1)
            bias_sb = cst_pool.tile([P, MT], dt.float32, name="bias_sb")
            nc.scalar.dma_start(bias_sb[:], bias[:, :])

            junk_mms(JUNK_PRE)

            # wave-0 weight slices (first from ScalarE so it's in flight
            # before any engine blocks)
            pwq = {}
            a0, b0 = CH[0]
            for m in range(J):
                pwq[(m, 0)] = prep_pw(
                    m, a0, b0,
                    dma_engine=nc.scalar if m == 0 else nc.gpsimd)
            quant_chain(xq_tiles[0][:, 0, :], xs0[:, 0, :], s_x, True)
            quant_chain(xq_tiles[0][:, 1, :], xs0[:, 1, :], s_x, False)
            quant_chain(xq_tiles[1][:, 0, :], xs1[:, 0, :], s_x, True)
            quant_chain(xq_tiles[1][:, 1, :], xs1[:, 1, :], s_x, False)

            partials = [part_pool.tile([P, TPC], dt.float32, name=f"pt{m}",
                                       tag=f"pt{m}") for m in range(J)]

            wqs = {}

            def ps_alloc():
                return [psA_pool.tile([P, TF], dt.float32, name="ps0"),
                        psB_pool.tile([P, TF], dt.float32, name="ps1")]

            # steady-state prefetch plan: wq[J], wq[J+1] built in quarter
            # k-slices spread over late waves (2 per wave, after
            # prologue-priority DMAs, riding the ~15% per-wave DMA slack)
            PREF = [(J + i, q * 8, q * 8 + 8) for i in range(2)
                    for q in range(4)]
            for mt, _, _ in PREF:
                if mt not in wqs:
                    wqs[mt] = wq_pool.tile([P, KT, P], dt.float16, name="wq",
                                           tag="wq")
            pref_i = 0

            for c, (a, b) in enumerate(CH):
                nxt = CH[c + 1] if c + 1 < NW else None
                # next chunk's x DMAs lead the wave's issue order
                pend_x = []
                if nxt is not None:
                    for kp in range(nxt[0] // 2, nxt[1] // 2):
                        pend_x.append((kp, issue_x_pair_dma(kp)))
                inj = INJ.get(len(pend_x), ())
                for m in range(J):
                    # this wave's matmul groups
                    wq_c = pwq.pop((m, c))
                    pss = ps_alloc()
                    for tf in range(NTF):
                        for kt in range(a, b):
                            nc.tensor.matmul(
                                pss[tf][:],
                                wq_c[:, kt - a, :],
                                xq_tiles[kt // 2][:, kt % 2,
                                                  tf * TF:(tf + 1) * TF],
                                start=(kt == a), stop=(kt == b - 1))
                    junk_mms(JW[c])
                    # spill/accumulate the partial (exact int in fp32);
                    # PSUM is only readable from DVE/ScalarE, and ScalarE
                    # can't add tensors -- adds ride DVE (~0.6us per
                    # bank, which is why chunks are >=4 k-tiles).  Spills
                    # go FIRST on DVE each step so PSUM slots free
                    # promptly.  Wave-0 copies split ScalarE/DVE.
                    for tf in range(NTF):
                        sl = partials[m][:, tf * TF:(tf + 1) * TF]
                        if c == 0:
                            if tf == 0:
                                nc.scalar.activation(sl, pss[tf][:], AF.Copy)
                            else:
                                nc.vector.tensor_copy(sl, pss[tf][:])
                        else:
                            nc.vector.tensor_tensor(sl, sl, pss[tf][:],
                                                    OP.add)
                    # next wave's weight slice for this m-tile.  All w
                    # DMAs ride the GpSimd queue so they never serialize
                    # behind the 4MB of x on Sync's queue.
                    if nxt is not None:
                        pwq[(m, c + 1)] = prep_pw(
                            m, nxt[0], nxt[1], dma_engine=nc.gpsimd)
                    # x chains injected mid-wave (their DMAs have landed
                    # by now); affines ride ScalarE except in wave 0
                    # where ScalarE carries the spill copies
                    if pend_x and m in inj:
                        kp, xs = pend_x.pop(0)
                        chain_x_pair(kp, xs, aff_scalar=(c > 0))
                    # steady wq prefetch rides late waves' DMA slack
                    # (issued on Sync behind the x pairs: lowest priority)
                    if c >= 1 and m == 7 and nxt is not None:
                        for _ in range(2):
                            if pref_i < len(PREF):
                                mt, sa, sb = PREF[pref_i]
                                prep_w_big_sub(wqs[mt], mt, sa, sb)
                                pref_i += 1
                while pend_x:
                    kp, xs = pend_x.pop(0)
                    chain_x_pair(kp, xs, aff_scalar=(c > 0))
                while c == NW - 1 and pref_i < len(PREF):
                    mt, sa, sb = PREF[pref_i]
                    prep_w_big_sub(wqs[mt], mt, sa, sb)
                    pref_i += 1

            # ---- steady-state m-loop, pipelined two m-tiles ahead ----
            # prologue stores (dequant from SBUF partials) are injected
            # one m-tile per steady iteration.
            for mt in range(J, MT):
                wq = wqs.pop(mt)
                if mt + 2 < MT:
                    wqs[mt + 2] = prep_w_big(mt + 2)
                pss = ps_alloc()
                last = (mt == MT - 1)
                if last:
                    # serialize banks so bank0's store hides under
                    # bank1's matmuls
                    for tf in range(NTF):
                        for kt in range(KT):
                            nc.tensor.matmul(
                                pss[tf][:], wq[:, kt, :],
                                xq_tiles[kt // 2][:, kt % 2,
                                                  tf * TF:(tf + 1) * TF],
                                start=(kt == 0), stop=(kt == KT - 1))
                        if tf == 0:
                            outt = out_pool.tile([P, TF], dt.float16,
                                                 name="outt")
                            nc.scalar.activation(
                                outt[:], pss[0][:], AF.Identity,
                                bias=bias_sb[:, mt:mt + 1], scale=inv_s)
                            nc.scalar.dma_start(
                                out[mt * P:(mt + 1) * P, 0:TF], outt[:])
                    # final half-split store on two engines
                    outt = out_pool.tile([P, TF], dt.float16, name="outt")
                    nc.scalar.activation(
                        outt[:], pss[1][:], AF.Identity,
                        bias=bias_sb[:, mt:mt + 1], scale=inv_s)
                    h = TF // 2
                    nc.sync.dma_start(
                        out[mt * P:(mt + 1) * P, TF + h:2 * TF],
                        outt[:, h:TF])
                    nc.scalar.dma_start(
                        out[mt * P:(mt + 1) * P, TF:TF + h], outt[:, 0:h])
                else:
                    for kt in range(KT):
                        for tf in range(NTF):
                            nc.tensor.matmul(
                                pss[tf][:], wq[:, kt, :],
                                xq_tiles[kt // 2][:, kt % 2,
                                                  tf * TF:(tf + 1) * TF],
                                start=(kt == 0), stop=(kt == KT - 1))
                    store(mt, pss)
                # prologue stores: one 512-token bank per steady
                # iteration, spread over the first 16 iterations
                pi = mt - J
                if pi < J * NTF:
                    pm, ptf = pi // NTF, pi % NTF
                    outt = out_pool.tile([P, TF], dt.float16, name="outt")
                    nc.scalar.activation(
                        outt[:], partials[pm][:, ptf * TF:(ptf + 1) * TF],
                        AF.Identity, bias=bias_sb[:, pm:pm + 1],
                        scale=inv_s)
                    nc.scalar.dma_start(
                        out[pm * P:(pm + 1) * P, ptf * TF:(ptf + 1) * TF],
                        outt[:])

    nc.compile()
    return nc


def _prep(x, weight, bias, amax_x, amax_w):
    ax = np.float32(np.asarray(amax_x, dtype=np.float32).reshape(-1)[0])
    aw = np.float32(np.asarray(amax_w, dtype=np.float32).reshape(-1)[0])
    s_x = np.float32(127.0) / ax
    s_w = np.float32(127.0) / aw
    inv_s = np.float32(1.0) / (s_x * s_w)

    x = np.asarray(x, dtype=np.float32)
    weight = np.asarray(weight, dtype=np.float32)
    bias = np.asarray(bias, dtype=np.float32)

    xT = np.ascontiguousarray(x.T)  # [K, N]
    # [MT, K, 128]: per m-tile a contiguous k-major block of W^T
    wt3 = np.ascontiguousarray(weight.reshape(MT, P, K).transpose(0, 2, 1))
    in_maps = [
        {
            "xt": np.ascontiguousarray(xT[:, c * TPC:(c + 1) * TPC]),
            "wt": wt3,
            "bias": np.ascontiguousarray(bias.reshape(MT, P).T),
        }
        for c in range(N_CORES)
    ]
    return float(s_x), float(s_w), float(inv_s), in_maps


def _spot_check(full, x, weight, bias, amax_x, amax_w, n=8):
    """Cheap host-side validation of a few output elements against the exact
    quantized-GEMM reference; catches transient device faults (observed as
    both exec errors and corrupted outputs on this fleet)."""
    rng = np.random.default_rng(0)
    ii = rng.integers(0, x.shape[0], size=n)
    jj = rng.integers(0, weight.shape[0], size=n)
    ax = np.float32(np.asarray(amax_x, np.float32).reshape(-1)[0])
    aw = np.float32(np.asarray(amax_w, np.float32).reshape(-1)[0])
    s_x = np.float32(127.0) / ax
    s_w = np.float32(127.0) / aw
    for i, j in zip(ii, jj):
        qx = np.clip(np.round(x[i].astype(np.float32) * s_x), -127, 127)
        qw = np.clip(np.round(weight[j].astype(np.float32) * s_w), -127, 127)
        exp = float(qx @ qw) / float(s_x * s_w) + float(bias[j])
        if abs(float(full[i, j]) - exp) > 1e-2 * max(1.0, abs(exp)):
            return False
    return True


def run(x, weight, bias, amax_x, amax_w, trace: bool = False):
    from concourse.bass_utils import run_bass_kernel_spmd

    s_x, s_w, inv_s, in_maps = _prep(x, weight, bias, amax_x, amax_w)
    nc = build(s_x, s_w, inv_s)
    full = None
    res = None
    err = None
    for attempt in range(3):
        try:
            res = run_bass_kernel_spmd(nc, in_maps,
                                       core_ids=list(range(N_CORES)),
                                       trace=trace)
            shards = [res.results[c]["out"] for c in range(N_CORES)]
            full = np.concatenate([s.T for s in shards],
                                  axis=0).astype(np.float32)
            if _spot_check(full, x, weight, bias, amax_x, amax_w):
                return full, res
        except Exception as e:  # transient NRT exec faults: retry
            err = e
    if full is not None:
        return full, res
    raise err


def kernel(x, weight, bias, amax_x, amax_w):
    full, _ = run(x, weight, bias, amax_x, amax_w, trace=False)
    return full


# revision 29
# speedup vs baseline: 1.0068x; 1.0068x over previous
"""AdaPT quantized linear (int8-exact via fp16 matmul) on 8 TRN2 NeuronCores.

Reference computes:
    qx = clip(round(x * 127/amax_x), -127, 127)        [N, K] int8
    qw = clip(round(w * 127/amax_w), -127, 127)        [M, K] int8
    out = (qx @ qw.T) / ((127/amax_x)*(127/amax_w)) + bias

Strategy: data-parallel over the 8192-token dim (1024 tokens/core), full
weight on every core, no collectives.  All int8 values are exactly
representable in fp16, the PE multiplies fp16 exactly (products < 2^14)
and accumulates in fp32 (partial sums << 2^24), so the fp16 matmul
reproduces the int8 systolic GEMM bit-exactly.

Rounding trick: fp16 has unit spacing on [1024, 2048), so converting
(x*scale + 1536) from fp32 to fp16 rounds the fractional part to the
nearest integer with ties-to-even -- exactly jnp.round.

The PE floor is one fp16 pass = 2048 matmuls x 216ns = 442us (fp8
DoubleRow measured at 2x per k-tile makes any exact 3-product split
1.5x slower -- not viable).  What this version optimizes is everything
around that floor:

  Prologue (chunked partial accumulation): while x streams in (16.8MB,
  ~47us at ~358GB/s), only 4 m-tiles can be PSUM-resident at full token
  width, which caps available PE work below the DMA deficit and used to
  idle the PE ~25us (and drop its pstate clock, doubling matmul time
  for stretches).  Instead the first J=8 m-tiles accumulate k-CHUNKS:
  when x chunk c arrives, each prologue m-tile runs just those k-tiles
  and the PSUM partial (an exact integer in fp32) is spilled/added to
  an SBUF accumulator by the DVE, freeing the bank immediately.  With
  J=8 the per-k-tile balance is PE 3.45us vs DMA 2.93us -- the PE
  stays busy for the whole x stream.  Weight slices for prologue
  m-tiles arrive chunk-by-chunk and die after one wave, so SBUF holds
  only ~1KB/partition of prologue weights at a time.

  Steady state (m-tiles J..31): classic pipeline, weight prep 2 m-tiles
  ahead, unchanged from the previous version.

  Tail: the last m-tile runs its two 512-token banks serially so the
  first bank's dequant+store hides under the second bank's matmuls.

Output is stored as fp16 ([M, tokens/core] transposed per core, so the
bias rides the ScalarE per-partition bias port) and upcast on the host;
the ~3e-4 relative rounding is far inside the 2e-2 gate.
"""

import sys

import numpy as np

sys.path.insert(0, "/opt/trn_rl_repo")

N, K, M = 8192, 4096, 4096
N_CORES = 8
TPC = N // N_CORES  # tokens per core
P = 128
KT = K // P   # 32 k-tiles
MT = M // P   # 32 m-tiles
TF = 512      # matmul moving free dim (one PSUM bank of fp32)
NTF = TPC // TF
MAGIC = 1536.0   # fp16 unit-spacing window base: round via fp32->fp16 convert
MAXV = 127.0
HI = MAGIC + MAXV
LO = MAGIC - MAXV
JUNK_PRE = 32    # PE pstate-ramp warmup matmuls before first real work

J = 8            # prologue m-tiles (chunk-partial accumulation)
# k-tile chunks per prologue wave (pair-aligned).  All 8 k-tiles: the
# 16 PSUM spills per wave are the scarce resource (only DVE can add
# from PSUM, ~0.6us per bank), so fewer, bigger waves.
CH = [(0, 8), (8, 16), (16, 24), (24, 32)]
NW = len(CH)
# junk plugs after each group in early waves (DMA still catching up)
JW = [1, 0, 0, 0]
# x-chain injection step within a wave, by pending pair count
INJ = {1: (4,), 2: (2, 5), 3: (1, 3, 6), 4: (1, 3, 5, 7)}


def build(s_x: float, s_w: float, inv_s: float):
    import concourse.mybir as mybir
    import concourse.tile as tile
    from concourse import bacc

    dt = mybir.dt
    AF = mybir.ActivationFunctionType
    OP = mybir.AluOpType

    nc = bacc.Bacc("TRN2", target_bir_lowering=False, debug=False,
                   num_devices=N_CORES)

    xt = nc.declare_dram_parameter("xt", [K, TPC], dt.float32, isOutput=False)
    wt = nc.declare_dram_parameter("wt", [MT, K, P], dt.float32, isOutput=False)
    # bias pre-transposed to [P, MT] on the host: the [M]->(p,o) gather
    # was a 4096x4B strided DMA that stalled its issuing queue for 11us
    bias = nc.declare_dram_parameter("bias", [P, MT], dt.float32,
                                     isOutput=False)
    out = nc.declare_dram_parameter("out", [M, TPC], dt.float16, isOutput=True)

    with tile.TileContext(nc) as tc:
        with (
            tc.tile_pool(name="xq", bufs=1) as xq_pool,
            tc.tile_pool(name="xs", bufs=3) as xs_pool,
            tc.tile_pool(name="pwq", bufs=10) as pwq_pool,
            tc.tile_pool(name="pws", bufs=4) as pws_pool,
            tc.tile_pool(name="part", bufs=1) as part_pool,
            tc.tile_pool(name="scr", bufs=3) as scr_pool,
            tc.tile_pool(name="wsb", bufs=2) as wsb_pool,
            tc.tile_pool(name="wq", bufs=3) as wq_pool,
            tc.tile_pool(name="cst", bufs=1) as cst_pool,
            tc.tile_pool(name="outp", bufs=4) as out_pool,
            tc.tile_pool(name="psA", bufs=4, space="PSUM") as psA_pool,
            tc.tile_pool(name="psB", bufs=3, space="PSUM") as psB_pool,
            tc.tile_pool(name="junk", bufs=1, space="PSUM") as junk_psum_pool,
        ):
            # ---- junk machinery (PE pstate keep-warm) ----------------
            # Junk matmuls write a dedicated PSUM bank that is never
            # read; the input lives outside tile tracking so the first
            # warmup matmuls don't wait on the memset.
            junk_t = nc.alloc_sbuf_tensor("junk_sb", [P, TF], dt.float16)
            junk_sb = junk_t.ap()
            junk_ps = junk_psum_pool.tile([P, TF], dt.float32, name="jps")

            def junk_mms(n):
                for _ in range(n):
                    nc.tensor.matmul(junk_ps[:], junk_sb[:, :P], junk_sb[:],
                                     start=True, stop=False,
                                     skip_group_check=True)

            xq_tiles = [
                xq_pool.tile([P, 2, TPC], dt.float16, name=f"xq{p}",
                             tag=f"xq{p}")
                for p in range(KT // 2)
            ]

            def quant_chain(dst, src, scale, aff_scalar):
                # dst fp16 = clip(round(src*scale), -127, 127) + 1536,
                # recentred; the affine's fp32->fp16 convert is the round.
                if aff_scalar:
                    nc.scalar.activation(dst, src, AF.Copy,
                                         bias=MAGIC, scale=scale)
                else:
                    nc.vector.tensor_scalar(dst, src, scale, MAGIC,
                                            OP.mult, OP.add)
                nc.vector.tensor_scalar(dst, dst, HI, LO, OP.min, OP.max)
                nc.vector.tensor_scalar(dst, dst, MAGIC, None, OP.subtract)

            def issue_x_pair_dma(kp):
                # per-k-tile DMAs (0.5MB arrival quantum)
                kt0 = 2 * kp
                xs = xs_pool.tile([P, 2, TPC], dt.float32, name="xs")
                for h in range(2):
                    nc.sync.dma_start(
                        xs[:, h, :],
                        xt[(kt0 + h) * P:(kt0 + h + 1) * P, :]
                        .rearrange("(o p) t -> p o t", p=P))
                return xs

            def chain_x_pair(kp, xs, aff_scalar=True):
                for h in range(2):
                    quant_chain(xq_tiles[kp][:, h, :], xs[:, h, :], s_x,
                                aff_scalar=aff_scalar)

            # ---- prologue weight slices (one wave's k-chunk, one m-tile)
            def prep_pw(mt, a, b, dma_engine=None):
                ws = pws_pool.tile([P, 8, P], dt.float32, name="pws")
                wqs_ = pwq_pool.tile([P, 8, P], dt.float16, name="pwq")
                (dma_engine or nc.sync).dma_start(
                    ws[:, :b - a, :],
                    wt[mt, a * P:b * P, :]
                    .rearrange("(o p) f -> p o f", p=P))
                # affine on ScalarE (prologue DVE is loaded with spills)
                nc.scalar.activation(wqs_[:, :b - a, :], ws[:, :b - a, :],
                                     AF.Copy, bias=MAGIC, scale=s_w)
                nc.vector.tensor_scalar(wqs_[:, :b - a, :], wqs_[:, :b - a, :],
                                        HI, LO, OP.min, OP.max)
                nc.vector.tensor_scalar(wqs_[:, :b - a, :], wqs_[:, :b - a, :],
                                        MAGIC, None, OP.subtract)
                return wqs_

            def prep_w_big(mt):
                # steady state: 2 half DMAs + 2 half chains on DVE
                wq = wq_pool.tile([P, KT, P], dt.float16, name="wq", tag="wq")
                h = KT // 2
                for q in range(2):
                    ws = wsb_pool.tile([P, KT // 2, P], dt.float32,
                                       name="wsb")
                    nc.sync.dma_start(
                        ws[:],
                        wt[mt, q * h * P:(q + 1) * h * P, :]
                        .rearrange("(o p) f -> p o f", p=P))
                    quant_chain(wq[:, q * h:(q + 1) * h, :], ws[:],
                                s_w, aff_scalar=False)
                return wq

            def prep_w_big_sub(wq, mt, a, b, dma_engine=None):
                # one k-slice of a steady wq tile (prefetch during prologue)
                ws = wsb_pool.tile([P, KT // 2, P], dt.float32, name="wsb")
                (dma_engine or nc.sync).dma_start(
                    ws[:, :b - a, :],
                    wt[mt, a * P:b * P, :]
                    .rearrange("(o p) f -> p o f", p=P))
                quant_chain(wq[:, a:b, :], ws[:, :b - a, :], s_w,
                            aff_scalar=False)

            def store(mt, src_banks, last=False):
                # dequant on ScalarE; out DMA issued from ScalarE too.
                for tf in range(NTF):
                    outt = out_pool.tile([P, TF], dt.float16, name="outt")
                    nc.scalar.activation(
                        outt[:], src_banks[tf][:],
                        AF.Identity, bias=bias_sb[:, mt:mt + 1],
                        scale=inv_s)
                    if last and tf == NTF - 1:
                        h = TF // 2
                        nc.sync.dma_start(
                            out[mt * P:(mt + 1) * P,
                                tf * TF + h:(tf + 1) * TF],
                            outt[:, h:TF])
                        nc.scalar.dma_start(
                            out[mt * P:(mt + 1) * P,
                                tf * TF:tf * TF + h],
                            outt[:, 0:h])
                    else:
                        nc.scalar.dma_start(
                            out[mt * P:(mt + 1) * P,
                                tf * TF:(tf + 1) * TF],
                            outt[:])

            # ---- prologue ---------------------------------------------
            # bias + chunk-0 x pairs + wave-0 weight slices in flight first
            xs0 = xs_pool.tile([P, 2, TPC], dt.float32, name="xs")
            nc.sync.dma_start(
                xs0[:, 0, :], xt[0:P, :].rearrange("(o p) t -> p o t", p=P))
            nc.vector.memset(junk_sb, 0.0)
            nc.sync.dma_start(
                xs0[:, 1, :], xt[P:2 * P, :].rearrange("(o p) t -> p o t", p=P))
            xs1 = issue_x_pair_dma(1)
            bias_sb = cst_pool.tile([P, MT], dt.float32, name="bias_sb")
            nc.scalar.dma_start(bias_sb[:], bias[:, :])

            junk_mms(JUNK_PRE)

            # wave-0 weight slices (first from ScalarE so it's in flight
            # before any engine blocks)
            pwq = {}
            a0, b0 = CH[0]
            for m in range(J):
                pwq[(m, 0)] = prep_pw(
                    m, a0, b0,
                    dma_engine=nc.scalar if m == 0 else nc.gpsimd)
            quant_chain(xq_tiles[0][:, 0, :], xs0[:, 0, :], s_x, True)
            quant_chain(xq_tiles[0][:, 1, :], xs0[:, 1, :], s_x, False)
            quant_chain(xq_tiles[1][:, 0, :], xs1[:, 0, :], s_x, True)
            quant_chain(xq_tiles[1][:, 1, :], xs1[:, 1, :], s_x, False)
            xs2 = issue_x_pair_dma(2)
            xs3 = issue_x_pair_dma(3)
            quant_chain(xq_tiles[2][:, 0, :], xs2[:, 0, :], s_x, True)
            quant_chain(xq_tiles[2][:, 1, :], xs2[:, 1, :], s_x, False)
            quant_chain(xq_tiles[3][:, 0, :], xs3[:, 0, :], s_x, True)
            quant_chain(xq_tiles[3][:, 1, :], xs3[:, 1, :], s_x, False)

            partials = [part_pool.tile([P, TPC], dt.float32, name=f"pt{m}",
                                       tag=f"pt{m}") for m in range(J)]

            wqs = {}

            def ps_alloc():
                return [psA_pool.tile([P, TF], dt.float32, name="ps0"),
                        psB_pool.tile([P, TF], dt.float32, name="ps1")]

            # steady-state prefetch plan: wq[J], wq[J+1] built in quarter
            # k-slices spread over late waves (2 per wave, after
            # prologue-priority DMAs, riding the ~15% per-wave DMA slack)
            PREF = [(J + i, q * 8, q * 8 + 8) for i in range(2)
                    for q in range(4)]
            for mt, _, _ in PREF:
                if mt not in wqs:
                    wqs[mt] = wq_pool.tile([P, KT, P], dt.float16, name="wq",
                                           tag="wq")
            pref_i = 0

            for c, (a, b) in enumerate(CH):
                nxt = CH[c + 1] if c + 1 < NW else None
                # next chunk's x DMAs lead the wave's issue order
                pend_x = []
                if nxt is not None:
                    for kp in range(nxt[0] // 2, nxt[1] // 2):
                        pend_x.append((kp, issue_x_pair_dma(kp)))
                inj = INJ.get(len(pend_x), ())
                for m in range(J):
                    # this wave's matmul groups
                    wq_c = pwq.pop((m, c))
                    pss = ps_alloc()
                    for tf in range(NTF):
                        for kt in range(a, b):
                            nc.tensor.matmul(
                                pss[tf][:],
                                wq_c[:, kt - a, :],
                                xq_tiles[kt // 2][:, kt % 2,
                                                  tf * TF:(tf + 1) * TF],
                                start=(kt == a), stop=(kt == b - 1))
                    junk_mms(JW[c])
                    # spill/accumulate the partial (exact int in fp32).
                    # Only DVE/ScalarE can read PSUM, only DVE/GpSimd
                    # can add tensors, so the two banks split:
                    #   tf0: ScalarE copy PSUM->scratch, GpSimd adds
                    #        scratch into the partial (SBUF only)
                    #   tf1: DVE adds straight from PSUM
                    sl0 = partials[m][:, 0:TF]
                    if c == 0:
                        nc.scalar.activation(sl0, pss[0][:], AF.Copy)
                    else:
                        scr = scr_pool.tile([P, TF], dt.float32, name="scr")
                        nc.scalar.activation(scr[:], pss[0][:], AF.Copy)
                        nc.gpsimd.tensor_tensor(sl0, sl0, scr[:], OP.add)
                    sl1 = partials[m][:, TF:TPC]
                    if c == 0:
                        nc.vector.tensor_copy(sl1, pss[1][:])
                    else:
                        nc.vector.tensor_tensor(sl1, sl1, pss[1][:], OP.add)
                    # next wave's weight slice for this m-tile.  All w
                    # DMAs ride the GpSimd queue so they never serialize
                    # behind the 4MB of x on Sync's queue.
                    if nxt is not None:
                        pwq[(m, c + 1)] = prep_pw(
                            m, nxt[0], nxt[1], dma_engine=nc.gpsimd)
                    # x chains injected mid-wave (their DMAs have landed
                    # by now); affine + clips on DVE
                    if pend_x and m in inj:
                        kp, xs = pend_x.pop(0)
                        chain_x_pair(kp, xs, aff_scalar=False)
                    # steady wq prefetch rides the waves' DMA slack
                    # (issued on Sync behind the x pairs: lowest priority)
                    if m == 7 and nxt is not None:
                        for _ in range(2):
                            if pref_i < len(PREF):
                                mt, sa, sb = PREF[pref_i]
                                prep_w_big_sub(wqs[mt], mt, sa, sb)
                                pref_i += 1
                while pend_x:
                    kp, xs = pend_x.pop(0)
                    chain_x_pair(kp, xs, aff_scalar=False)
                while c == NW - 1 and pref_i < len(PREF):
                    mt, sa, sb = PREF[pref_i]
                    prep_w_big_sub(wqs[mt], mt, sa, sb)
                    pref_i += 1

            # ---- steady-state m-loop, pipelined two m-tiles ahead ----
            # prologue stores (dequant from SBUF partials) are injected
            # one m-tile per steady iteration.
            for mt in range(J, MT):
                wq = wqs.pop(mt)
                if mt + 2 < MT:
                    wqs[mt + 2] = prep_w_big(mt + 2)
                pss = ps_alloc()
                last = (mt == MT - 1)
                if last:
                    # serialize banks so bank0's store hides under
                    # bank1's matmuls
                    for tf in range(NTF):
                        for kt in range(KT):
                            nc.tensor.matmul(
                                pss[tf][:], wq[:, kt, :],
                                xq_tiles[kt // 2][:, kt % 2,
                                                  tf * TF:(tf + 1) * TF],
                                start=(kt == 0), stop=(kt == KT - 1))
                        if tf == 0:
                            outt = out_pool.tile([P, TF], dt.float16,
                                                 name="outt")
                            nc.scalar.activation(
                                outt[:], pss[0][:], AF.Identity,
                                bias=bias_sb[:, mt:mt + 1], scale=inv_s)
                            nc.scalar.dma_start(
                                out[mt * P:(mt + 1) * P, 0:TF], outt[:])
                    # final half-split store on two engines
                    outt = out_pool.tile([P, TF], dt.float16, name="outt")
                    nc.scalar.activation(
                        outt[:], pss[1][:], AF.Identity,
                        bias=bias_sb[:, mt:mt + 1], scale=inv_s)
                    h = TF // 2
                    nc.sync.dma_start(
                        out[mt * P:(mt + 1) * P, TF + h:2 * TF],
                        outt[:, h:TF])
                    nc.scalar.dma_start(
                        out[mt * P:(mt + 1) * P, TF:TF + h], outt[:, 0:h])
                else:
                    for kt in range(KT):
                        for tf in range(NTF):
                            nc.tensor.matmul(
                                pss[tf][:], wq[:, kt, :],
                                xq_tiles[kt // 2][:, kt % 2,
                                                  tf * TF:(tf + 1) * TF],
                                start=(kt == 0), stop=(kt == KT - 1))
                    store(mt, pss)
                # prologue stores: one 512-token bank per steady
                # iteration, spread over the first 16 iterations
                pi = mt - J
                if pi < J * NTF:
                    pm, ptf = pi // NTF, pi % NTF
                    outt = out_pool.tile([P, TF], dt.float16, name="outt")
                    nc.scalar.activation(
                        outt[:], partials[pm][:, ptf * TF:(ptf + 1) * TF],
                        AF.Identity, bias=bias_sb[:, pm:pm + 1],
                        scale=inv_s)
                    nc.scalar.dma_start(
                        out[pm * P:(pm + 1) * P, ptf * TF:(ptf + 1) * TF],
                        outt[:])

    nc.compile()
    return nc


def _prep(x, weight, bias, amax_x, amax_w):
    ax = np.float32(np.asarray(amax_x, dtype=np.float32).reshape(-1)[0])
    aw = np.float32(np.asarray(amax_w, dtype=np.float32).reshape(-1)[0])
    s_x = np.float32(127.0) / ax
    s_w = np.float32(127.0) / aw
    inv_s = np.float32(1.0) / (s_x * s_w)

    x = np.asarray(x, dtype=np.float32)
    weight = np.asarray(weight, dtype=np.float32)
    bias = np.asarray(bias, dtype=np.float32)

    xT = np.ascontiguousarray(x.T)  # [K, N]
    # [MT, K, 128]: per m-tile a contiguous k-major block of W^T
    wt3 = np.ascontiguousarray(weight.reshape(MT, P, K).transpose(0, 2, 1))
    in_maps = [
        {
            "xt": np.ascontiguousarray(xT[:, c * TPC:(c + 1) * TPC]),
            "wt": wt3,
            "bias": np.ascontiguousarray(bias.reshape(MT, P).T),
        }
        for c in range(N_CORES)
    ]
    return float(s_x), float(s_w), float(inv_s), in_maps


def _spot_check(full, x, weight, bias, amax_x, amax_w, n=8):
    """Cheap host-side validation of a few output elements against the exact
    quantized-GEMM reference; catches transient device faults (observed as
    both exec errors and corrupted outputs on this fleet)."""
    rng = np.random.default_rng(0)
    ii = rng.integers(0, x.shape[0], size=n)
    jj = rng.integers(0, weight.shape[0], size=n)
    ax = np.float32(np.asarray(amax_x, np.float32).reshape(-1)[0])
    aw = np.float32(np.asarray(amax_w, np.float32).reshape(-1)[0])
    s_x = np.float32(127.0) / ax
    s_w = np.float32(127.0) / aw
    for i, j in zip(ii, jj):
        qx = np.clip(np.round(x[i].astype(np.float32) * s_x), -127, 127)
        qw = np.clip(np.round(weight[j].astype(np.float32) * s_w), -127, 127)
        exp = float(qx @ qw) / float(s_x * s_w) + float(bias[j])
        if abs(float(full[i, j]) - exp) > 1e-2 * max(1.0, abs(exp)):
            return False
    return True


def run(x, weight, bias, amax_x, amax_w, trace: bool = False):
    from concourse.bass_utils import run_bass_kernel_spmd

    s_x, s_w, inv_s, in_maps = _prep(x, weight, bias, amax_x, amax_w)
    nc = build(s_x, s_w, inv_s)
    full = None
    res = None
    err = None
    for attempt in range(3):
        try:
            res = run_bass_kernel_spmd(nc, in_maps,
                                       core_ids=list(range(N_CORES)),
                                       trace=trace)
            shards = [res.results[c]["out"] for c in range(N_CORES)]
            full = np.concatenate([s.T for s in shards],
                                  axis=0).astype(np.float32)
            if _spot_check(full, x, weight, bias, amax_x, amax_w):
                return full, res
        except Exception as e:  # transient NRT exec faults: retry
            err = e
    if full is not None:
        return full, res
    raise err


def kernel(x, weight, bias, amax_x, amax_w):
    full, _ = run(x, weight, bias, amax_x, amax_w, trace=False)
    return full
